# revision 19
# baseline (speedup 1.0000x reference)
"""Trainium2 Bass kernel for the depth-dependent camera rendering problem.

Strategy (v2)
-------------
Host (numpy, float64): PSF synthesis (phase -> Hankel einsum -> radial
interp -> quadrant mirror -> fftshift -> normalize) and rfft2 of the PSF
(tiny: ~1% of FLOPs), plus input sharding.

Key identities vs v1:
  * The PSF is even about the half-pixel center, so
    Fpsf = R * exp(+i*pi*(kh+kw)/N) with R REAL.  The spectral multiply
    becomes one real elementwise product (fused with PSUM evacuation),
    and the conjugate phase is folded into half-pixel-shifted inverse
    DFT tables (host-side).
  * blur is linear and cum_d = sum_{d'>=d} lay_{d'}, so
    blur(cum_d) = sum_{d'>=d} blur(lay_{d'}) accumulates in the SPATIAL
    domain (cumb += alpha) -- the third inverse DFT per depth is gone.

Device (Bass/Tile, 8 NeuronCores, SPMD): 6 cores each own one (b, c)
chain.  Per core, a backward depth loop d = 15..0:
  layered_d = (idx == d); vol_d = layered_d * img/scale   (Pool engine)
  forward 2D DFT of both planes (matmul-only, no transposes)
  fa = Zlay * R[d];  fv = Zvol * R[d]      (DVE, reads PSUM directly)
  2 inverse 2D DFTs -> alpha, volb (left in PSUM)
  cumb += alpha;  rc = 1/(cumb+eps)
  acc = volb*rc + (1 - alpha*rc) * acc     (back-to-front over-composite)
Final acc = captimg[b, c] / scale.

2D DFT via chained matmuls (out = lhsT.T @ rhs contracts the partition
axis and swaps the other two), so the plane layout ping-pongs and no
transposes are ever needed:
  [H, W] --c1--> [W, hf(384)] --c2--> [hf, wf(193)]   (forward, rfft2 conv)
  [hf, wf] --iA--> [wf, H] --iB--> [H, W]             (inverse, shifted tables)
"""

import os
import time

import numpy as np

import concourse.bass as bass
import concourse.tile as tile
from concourse import bacc, mybir
from concourse.bass_utils import run_bass_kernel_spmd

dt = mybir.dt
Alu = mybir.AluOpType

# ---- problem constants (hardcoded; kernel.py must be self-contained) ----
N = 384            # image H = W
HF = N             # full spectrum bins along H
WF = N // 2 + 1    # rfft bins along W = 193
D = 16             # depth planes
B, C = 2, 3
EPS = 1e-3
NCORES = 8
WAVELENGTHS = np.array([632e-9, 550e-9, 450e-9])
FOCAL_LENGTH = 50e-3
FOCAL_DEPTH = 1.7
SENSOR_DIST = 1.0 / (1.0 / FOCAL_LENGTH - 1.0 / FOCAL_DEPTH)

MM_DT = dt.float32r   # matmul operand mode (full-rate); set dt.float32 for precision


# =====================================================================
# Host-side DFT tables
# =====================================================================
def _make_tables():
    k = np.arange(N, dtype=np.float64)
    th = 2.0 * np.pi * np.outer(k, k) / N     # [N, N]
    co = np.cos(th)
    sn = np.sin(th)
    # pass1 computes only the Hermitian half (kh 0..192); mirror columns are
    # reconstructed with reversed copies (conjugate symmetry of a real DFT).
    c1h = np.concatenate([co[:, :WF], -sn[:, :WF]], axis=1)     # [N, 2*WF]
    c2a = np.concatenate([co[:, :WF], -sn[:, :WF]], axis=1)     # [N, 2*WF]
    c2b = np.concatenate([sn[:, :WF], co[:, :WF]], axis=1)      # [N, 2*WF]
    # inverse tables with a half-pixel shift: they absorb the conjugate of
    # the PSF symmetry phase exp(-i*pi*(kh+kw)/N) folded into R.
    kh = np.arange(N).reshape(-1, 1)
    h = np.arange(N).reshape(1, -1)
    thA = 2.0 * np.pi * kh * (h + 0.5) / N
    caA = np.cos(thA)
    saA = np.sin(thA)
    ia = np.concatenate([caA, -saA], axis=1)                    # [N, 2N]
    kw = np.arange(WF).reshape(-1, 1)
    w = np.arange(N).reshape(1, -1)
    b = np.full(WF, 2.0)
    b[0] = 1.0
    b[WF - 1] = 1.0
    thB = 2.0 * np.pi * kw * (w + 0.5) / N
    ibr = b[:, None] * np.cos(thB)                              # [WF, N]
    ibi = -b[:, None] * np.sin(thB)                             # [WF, N]
    ib = np.stack([ibr, ibi], axis=0)                           # [2, WF, N]
    return (c1h.astype(np.float32), c2a.astype(np.float32),
            c2b.astype(np.float32), ia.astype(np.float32),
            saA.astype(np.float32), ib.astype(np.float32))


def _fwd_np(x, c1h, c2a, c2b):
    """Numpy mirror of the device forward DFT (for validation)."""
    x = x.astype(np.float32)
    yh = x.T @ c1h                                  # [W, 2*WF]: half spectrum
    y1r = np.concatenate([yh[:, :WF], yh[:, WF - 2:0:-1]], axis=1)     # [W, N]
    y1i = np.concatenate([yh[:, WF:], -yh[:, 2 * WF - 2:WF:-1]], axis=1)
    z = y1r.T @ c2a + y1i.T @ c2b                   # [HF, 2*WF]
    return z


def _inv_np(s, ia, sa, ib):
    """Numpy mirror of the device inverse DFT (for validation)."""
    sr, si = s[:, :WF], s[:, WF:]
    ca = ia[:, :N]
    nsa = ia[:, N:]
    pr = sr.T @ ca + si.T @ nsa                     # [WF, N]
    pi = sr.T @ sa + si.T @ ca                      # [WF, N]
    y = pr.T @ ib[0] + pi.T @ ib[1]                 # [N, N]
    return y


# =====================================================================
# Device program
# =====================================================================
def build_program(occlusion: bool, n_depth: int = D):
    nc = bacc.Bacc(None, target_bir_lowering=False, debug=False)
    f32 = dt.float32

    img_d = nc.declare_dram_parameter("img", [N, N], f32, isOutput=False)
    idx_d = nc.declare_dram_parameter("idx", [N, N], f32, isOutput=False)
    pf_d = nc.declare_dram_parameter("rpsf", [D, HF, 2 * WF], f32, isOutput=False)
    c1_d = nc.declare_dram_parameter("c1", [N, 2 * WF], MM_DT, isOutput=False)
    c2a_d = nc.declare_dram_parameter("c2a", [N, 2 * WF], MM_DT, isOutput=False)
    c2b_d = nc.declare_dram_parameter("c2b", [N, 2 * WF], MM_DT, isOutput=False)
    ia_d = nc.declare_dram_parameter("ia", [N, 2 * N], MM_DT, isOutput=False)
    is_d = nc.declare_dram_parameter("isa", [N, N], MM_DT, isOutput=False)
    ib_d = nc.declare_dram_parameter("ib", [2, WF, N], MM_DT, isOutput=False)
    out_d = nc.declare_dram_parameter("out", [N, N], f32, isOutput=True)

    PCH = [(0, 128), (128, 256), (256, 384)]          # partition chunks of 384
    WCH = [(0, 97), (97, WF)]                          # partition chunks of 193

    with tile.TileContext(nc) as tc:
        with (
            tc.tile_pool(name="const", bufs=1) as cp,
            tc.tile_pool(name="pers", bufs=1) as pp,
            tc.tile_pool(name="work", bufs=2) as wp,
            tc.tile_pool(name="spec", bufs=2) as sp,
            tc.tile_pool(name="y1p", bufs=2) as y1p,
            tc.tile_pool(name="pbp", bufs=2) as pbp,
            tc.tile_pool(name="pfp", bufs=2) as pfp,
            tc.tile_pool(name="psy1", bufs=2, space="PSUM") as ps_y1,
            tc.tile_pool(name="psz", bufs=2, space="PSUM") as ps_z,
            tc.tile_pool(name="pspr", bufs=1, space="PSUM") as ps_pr,
            tc.tile_pool(name="pspi", bufs=1, space="PSUM") as ps_pi,
            tc.tile_pool(name="psy", bufs=1, space="PSUM") as ps_y,
        ):
            # ---- load constants ----
            # Stage DMAs so the critical path (idx -> lay gen -> pass1 via c1)
            # is ready ASAP, spread across four DGE queues.
            def load3(dram, cols, tag, eng, dtype=f32):
                ts = []
                for ci, (lo, hi) in enumerate(PCH):
                    t = cp.tile([128, cols], dtype, name=f"{tag}{ci}", tag=f"{tag}{ci}")
                    eng.dma_start(t[:], dram[lo:hi, :])
                    ts.append(t)
                return ts

            idxt = load3(idx_d, N, "idx", nc.sync)
            c1t = load3(c1_d, 2 * WF, "c1", nc.scalar, MM_DT)  # fwd pass1 (half bins)
            imgt = load3(img_d, N, "img", nc.gpsimd)
            c2at = load3(c2a_d, 2 * WF, "c2a", nc.scalar, MM_DT)
            c2bt = load3(c2b_d, 2 * WF, "c2b", nc.scalar, MM_DT)
            iat = load3(ia_d, 2 * N, "ia", nc.gpsimd, MM_DT)  # inv stepA: [ca | -sa] (shifted)
            ist = load3(is_d, N, "isa", nc.gpsimd, MM_DT)     # inv stepA: sa (shifted)
            ibt = []                            # ib chunks: [2][wf-chunk]
            for comp in range(2):
                row = []
                for ci, (lo, hi) in enumerate(WCH):
                    t = cp.tile([hi - lo, N], MM_DT, name=f"ib{comp}{ci}", tag=f"ib{comp}{ci}")
                    nc.gpsimd.dma_start(t[:], ib_d[comp, lo:hi, :])
                    row.append(t)
                ibt.append(row)

            # persistent accumulators
            acct = [pp.tile([128, N], f32, name=f"acc{ci}", tag=f"acc{ci}") for ci in range(3)]
            cumt = [pp.tile([128, N], f32, name=f"cum{ci}", tag=f"cum{ci}") for ci in range(3)]
            # freq-domain accumulator for the non-occlusion path
            if not occlusion:
                fsum = [pp.tile([128, 2 * WF], f32, name=f"fs{ci}", tag=f"fs{ci}") for ci in range(3)]

            # ---------------- helpers ----------------
            def fwd_pass1(x3, name):
                """x3: 3 tiles [128, N] ([H, W]) -> y1: 3 tiles [128, 2N]
                (full kh spectrum, mirror half via Hermitian symmetry)."""
                y1 = [y1p.tile([128, 2 * N], MM_DT, name=f"y1{name}_{m}", tag=f"y1{name}_{m}") for m in range(3)]
                for m in range(3):
                    ph = ps_y1.tile([128, 2 * WF], f32, name="y1h", tag="y1h")
                    for k in range(3):
                        nc.tensor.matmul(
                            ph[:], x3[k][:, m * 128:(m + 1) * 128],
                            c1t[k][:],
                            start=(k == 0), stop=(k == 2))
                    # Hermitian reconstruction: y1[w, 384-kh] = conj(y1[w, kh])
                    nc.any.tensor_copy(y1[m][:, 0:WF], ph[:, 0:WF])
                    nc.any.tensor_copy(y1[m][:, WF:N], ph[:, WF - 2:0:-1])
                    nc.any.tensor_copy(y1[m][:, N:N + WF], ph[:, WF:2 * WF])
                    nc.any.tensor_scalar(
                        y1[m][:, N + WF:2 * N], ph[:, 2 * WF - 2:WF:-1],
                        -1.0, None, op0=Alu.mult)
                return y1

            def fwd_mul(x3, rt, name):
                """Single-field forward + spectral multiply (non-occlusion)."""
                y1 = fwd_pass1(x3, name)
                f = [sp.tile([128, 2 * WF], MM_DT, name=f"f_{name}{m}", tag=f"f_{name}{m}") for m in range(3)]
                for m in range(3):
                    pz = ps_z.tile([128, 2 * WF], f32, name="pz", tag="pz")
                    for k in range(3):
                        nc.tensor.matmul(
                            pz[:], y1[k][:, m * 128:(m + 1) * 128],
                            c2at[k][:],
                            start=(k == 0), stop=False)
                        nc.tensor.matmul(
                            pz[:], y1[k][:, N + m * 128:N + (m + 1) * 128],
                            c2bt[k][:],
                            start=False, stop=(k == 2))
                    # fused PSUM evacuation + real spectral multiply
                    nc.vector.tensor_mul(f[m][:], pz[:], rt[m][:])
                return f

            def fwd_mul2(laya, volv, rt):
                """Both fields: pass1 x2, then pass2 m-groups interleaved
                across fields so the pz PSUM bank reuse distance doubles."""
                y1a = fwd_pass1(laya, "a")
                y1v = fwd_pass1(volv, "v")
                fa = [sp.tile([128, 2 * WF], MM_DT, name=f"f_a{m}", tag=f"f_a{m}") for m in range(3)]
                fv = [sp.tile([128, 2 * WF], MM_DT, name=f"f_v{m}", tag=f"f_v{m}") for m in range(3)]
                for m in range(3):
                    for y1, f in ((y1a, fa), (y1v, fv)):
                        pz = ps_z.tile([128, 2 * WF], f32, name="pz", tag="pz")
                        for k in range(3):
                            nc.tensor.matmul(
                                pz[:], y1[k][:, m * 128:(m + 1) * 128],
                                c2at[k][:],
                                start=(k == 0), stop=False)
                            nc.tensor.matmul(
                                pz[:], y1[k][:, N + m * 128:N + (m + 1) * 128],
                                c2bt[k][:],
                                start=False, stop=(k == 2))
                        # fused PSUM evacuation + real spectral multiply
                        nc.vector.tensor_mul(f[m][:], pz[:], rt[m][:])
                return fa, fv

            def inv(f3, name):
                """f3: 3 SBUF tiles [128, 2*WF] -> y: 3 PSUM tiles [128, N]."""
                pch = []
                for mi, (lo, hi) in enumerate(WCH):
                    w = hi - lo
                    t = pbp.tile([w, 2 * N], MM_DT, name=f"p_{mi}", tag=f"p_{mi}")
                    prr = ps_pr.tile([97, N], f32, name="ppr", tag="ppr")
                    pii = ps_pi.tile([97, N], f32, name="ppi", tag="ppi")
                    for k in range(3):
                        # Pr = Sr.T @ ca + Si.T @ (-sa)
                        nc.tensor.matmul(
                            prr[:w], f3[k][:, lo:hi],
                            iat[k][:, 0:N],
                            start=(k == 0), stop=False)
                        nc.tensor.matmul(
                            prr[:w], f3[k][:, WF + lo:WF + hi],
                            iat[k][:, N:2 * N],
                            start=False, stop=(k == 2))
                        # Pi = Sr.T @ sa + Si.T @ ca
                        nc.tensor.matmul(
                            pii[:w], f3[k][:, lo:hi],
                            ist[k][:],
                            start=(k == 0), stop=False)
                        nc.tensor.matmul(
                            pii[:w], f3[k][:, WF + lo:WF + hi],
                            iat[k][:, 0:N],
                            start=False, stop=(k == 2))
                    nc.any.tensor_copy(t[:, 0:N], prr[:w])
                    nc.any.tensor_copy(t[:, N:2 * N], pii[:w])
                    pch.append(t)
                y = []
                for m in range(3):
                    py = ps_y.tile([128, N], f32, name=f"py_{name}", tag=f"py_{name}")
                    for k, (lo, hi) in enumerate(WCH):
                        w = hi - lo
                        nc.tensor.matmul(
                            py[:], pch[k][:w, m * 128:(m + 1) * 128],
                            ibt[0][k][:],
                            start=(k == 0), stop=False)
                        nc.tensor.matmul(
                            py[:], pch[k][:w, N + m * 128:N + (m + 1) * 128],
                            ibt[1][k][:],
                            start=False, stop=(k == 1))
                    # evacuate promptly so the single PSUM bank cycles
                    yt = wp.tile([128, N], f32, name=f"s{name}{m}", tag=f"s{name}{m}")
                    nc.any.tensor_copy(yt[:], py[:])
                    y.append(yt)
                return y

            # ---------------- main depth loop (back to front) ----------------
            def load_R(dd):
                rt = []
                for ci, (lo, hi) in enumerate(PCH):
                    t = pfp.tile([128, 2 * WF], f32, name=f"rt{ci}", tag=f"rt{ci}")
                    nc.sync.dma_start(t[:], pf_d[dd, lo:hi, :])
                    rt.append(t)
                return rt

            def gen_layvol(dd):
                # layered & volume planes (Pool engine: keep DVE free)
                lay = [wp.tile([128, N], MM_DT, name=f"lay{ci}", tag=f"lay{ci}") for ci in range(3)]
                vol = [wp.tile([128, N], MM_DT, name=f"vol{ci}", tag=f"vol{ci}") for ci in range(3)]
                for ci in range(3):
                    nc.gpsimd.tensor_scalar(
                        lay[ci][:], idxt[ci][:], float(dd), None, op0=Alu.is_equal)
                    nc.gpsimd.tensor_mul(vol[ci][:], lay[ci][:], imgt[ci][:])
                return lay, vol

            lay, vol = gen_layvol(n_depth - 1)
            rt = load_R(n_depth - 1)
            for dd in range(n_depth - 1, -1, -1):
                first = (dd == n_depth - 1)
                nxt = None
                if occlusion:
                    fa, fv = fwd_mul2(lay, vol, rt)
                    if dd > 0:
                        nxt = gen_layvol(dd - 1)
                        rt_n = load_R(dd - 1)
                    alpha = inv(fa, "a")
                    volb = inv(fv, "v")
                    for ci in range(3):
                        # cumb += alpha  (reads PSUM)
                        if first:
                            nc.vector.tensor_copy(cumt[ci][:], alpha[ci][:])
                        else:
                            nc.vector.tensor_add(cumt[ci][:], cumt[ci][:], alpha[ci][:])
                        rc = wp.tile([128, N], f32, name="rc", tag="rc")
                        nc.scalar.activation(
                            rc[:], cumt[ci][:], mybir.ActivationFunctionType.Copy,
                            bias=EPS)
                        nc.vector.reciprocal(rc[:], rc[:])
                        t1 = wp.tile([128, N], f32, name="t1", tag="t1")
                        if first:
                            # acc = volb * rc
                            nc.vector.tensor_mul(acct[ci][:], volb[ci][:], rc[:])
                        else:
                            # acc = (volb - alpha*acc) * rc + acc
                            nc.vector.tensor_mul(t1[:], alpha[ci][:], acct[ci][:])
                            nc.vector.tensor_sub(t1[:], volb[ci][:], t1[:])
                            nc.gpsimd.tensor_mul(t1[:], t1[:], rc[:])
                            nc.gpsimd.tensor_add(acct[ci][:], acct[ci][:], t1[:])
                else:
                    fv = fwd_mul(vol, rt, "v")
                    if dd > 0:
                        nxt = gen_layvol(dd - 1)
                        rt_n = load_R(dd - 1)
                    for ci in range(3):
                        if first:
                            nc.vector.tensor_copy(fsum[ci][:], fv[ci][:])
                        else:
                            nc.vector.tensor_add(fsum[ci][:], fsum[ci][:], fv[ci][:])
                if nxt is not None:
                    lay, vol = nxt
                    rt = rt_n

            if not occlusion:
                fmm = [sp.tile([128, 2 * WF], MM_DT, name=f"fm{ci}", tag=f"fm{ci}") for ci in range(3)]
                for ci in range(3):
                    nc.vector.tensor_copy(fmm[ci][:], fsum[ci][:])
                acc2 = inv(fmm, "f")
                for ci in range(3):
                    nc.vector.tensor_copy(acct[ci][:], acc2[ci][:])

            # store
            for ci, (lo, hi) in enumerate(PCH):
                nc.sync.dma_start(out_d[lo:hi, :], acct[ci][:])

    nc.compile()
    return nc


# =====================================================================
# Host-side PSF pipeline (float64, mirrors reference.py exactly)
# =====================================================================
def _host_psf(heightmap1d, prop_amplitude, prop_phase, H, rho_grid, rho_sampling):
    wl = WAVELENGTHS.reshape(3, 1, 1)
    hm = np.asarray(heightmap1d, np.float64).reshape(1, 1, -1)
    pa = np.asarray(prop_amplitude, np.float64)
    pp_ = np.asarray(prop_phase, np.float64)
    Hm = np.asarray(H, np.float64)
    rg = np.asarray(rho_grid, np.float64)
    rs = np.asarray(rho_sampling, np.float64)

    n_idx = 1.5375 + 0.00829045 / (wl * 1e6) ** 2 - 0.000211046 / (wl * 1e6) ** 4
    phase = 2.0 * np.pi / wl * (n_idx - 1.0) * hm + pp_          # [3,D,M]
    real = np.einsum('wdm,wmr->wdr', pa * np.cos(phase), Hm)
    imag = np.einsum('wdm,wmr->wdr', pa * np.sin(phase), Hm)
    psf1d = (2.0 * np.pi / (wl * SENSOR_DIST)) ** 2 * (real ** 2 + imag ** 2)

    hh = N // 2
    nd = psf1d.shape[1]
    psf_rd = np.empty((3, nd, hh * hh), np.float64)
    for w in range(3):
        sflat = rs[w].reshape(-1)
        for d in range(nd):
            psf_rd[w, d] = np.interp(sflat, rg[w], psf1d[w, d])
    psf_rd = np.maximum(psf_rd, 0.0).astype(np.float32).reshape(3, nd, hh, hh)
    q = np.concatenate([psf_rd[:, :, ::-1, :], psf_rd], axis=-2)
    psf = np.concatenate([q[:, :, :, ::-1], q], axis=-1)          # [3,D,N,N]
    psf = np.fft.fftshift(psf, axes=(-2, -1))
    psf = psf / np.sum(psf, axis=(-2, -1), keepdims=True)
    Fpsf = np.fft.rfft2(psf.astype(np.float64)) / float(N * N)    # [3,D,N,WF]
    # PSF is even about the half-pixel center: Fpsf = R * exp(i*pi*(kh+kw)/N)
    # with R real.  Fold the phase out; the inverse DFT tables carry it.
    kh = np.arange(N).reshape(-1, 1)
    kw = np.arange(WF).reshape(1, -1)
    ph = np.exp(-1j * np.pi * (kh + kw) / N)
    R = (Fpsf * ph).real                                          # [3,D,N,WF]
    Rd = np.concatenate([R, R], axis=-1).astype(np.float32)       # [3,D,N,2*WF]
    return np.ascontiguousarray(Rd)


_PROG_CACHE = {}


def kernel(img, depthmap, heightmap1d, prop_amplitude, prop_phase, H,
           rho_grid, rho_sampling, occlusion):
    occ = bool(np.asarray(occlusion).item())
    img = np.asarray(img, np.float32)
    depthmap = np.asarray(depthmap, np.float32)

    pf = _host_psf(heightmap1d, prop_amplitude, prop_phase, H, rho_grid, rho_sampling)

    scale = np.float32(img.max())
    imgs = img / scale                                            # [B,C,N,N] f32
    idxf = np.clip(np.floor(depthmap * np.float32(D)), 0, D - 1)[:, 0]  # [B,N,N]
    c1, c2a, c2b, ia, sa, ib = _make_tables()

    if occ not in _PROG_CACHE:
        _PROG_CACHE[occ] = build_program(occ)
    nc = _PROG_CACHE[occ]

    in_maps = []
    for core in range(NCORES):
        b_, c_ = divmod(core, C) if core < B * C else (0, 0)
        in_maps.append({
            "img": np.ascontiguousarray(imgs[b_, c_]),
            "idx": np.ascontiguousarray(idxf[b_]),
            "rpsf": np.ascontiguousarray(pf[c_]),
            "c1": c1, "c2a": c2a, "c2b": c2b, "ia": ia, "isa": sa, "ib": ib,
        })
    t0 = time.perf_counter()
    res_obj = run_bass_kernel_spmd(
        nc, in_maps, list(range(NCORES)),
        trace=bool(os.environ.get("KBASS_TRACE")))
    global LAST
    LAST = {"wall_s": time.perf_counter() - t0,
            "exec_time_ns": res_obj.exec_time_ns,
            "profile_json": res_obj.profile_json}
    res = res_obj.results
    out = np.empty((B, C, N, N), np.float32)
    for core in range(B * C):
        b_, c_ = divmod(core, C)
        out[b_, c_] = res[core]["out"] * scale
    return out
